# revision 5
# baseline (speedup 1.0000x reference)
"""GQA attention (B=2, S=2048, D=1024, H=16, KV=4, HD=64) with RoPE + causal
softmax + output projection, sharded over 8 trn2 NeuronCores.

Sharding: core c -> (b = c // 4, g = c % 4): one batch x one KV group
(4 query heads + 1 kv head) per core.  Wq/Wk/Wv column-sharded, Wo
row-sharded; the all-reduce over Wo row-shards is done on the host
(partial [S, D] outputs summed per batch).

Device kernel (per core, all matmuls bf16 with fp32 PSUM accumulate):
  1. QKV^T = Wqkv^T @ x  via lhsT=Wqkv-chunks, rhs=x^T-chunks (x^T is
     pre-transposed + bf16-cast on host).
  2. RoPE applied in the transposed (head-dim on partitions) layout via
     32-partition-slab multiplies against host-prepared cos/sin tables.
  3. scores^T computed directly (lhsT=k^T chunk, rhs=q^T) so softmax
     needs no transposes: exp on ACT (no max subtraction needed; inputs
     are scaled so |scores| is small), causal handled by trimming +
     a [128,128] additive mask on diagonal blocks.
  4. A^T = V_aug^T-style matmul with lhsT=[V | ones] so the softmax
     denominator accumulates into partitions 64:128 for free.
  5. normalize with reciprocal_approx_fast, then out = A @ Wo chunk,
     DMA'd straight from PSUM to DRAM as the fp32 partial output.
"""

import sys

if "/opt/trn_rl_repo" not in sys.path:
    sys.path.insert(0, "/opt/trn_rl_repo")

import numpy as np
import ml_dtypes

B, S, D = 2, 2048, 1024
H, KV, HD = 16, 4, 64
NHC = H // KV          # query heads per core = 4
FQ = NHC * HD          # 256
SPAN = 512
NSPAN = S // SPAN      # 4
NCHUNK = S // 128      # 16
KD = D // 128          # 8
BF16 = ml_dtypes.bfloat16

_CACHE = {}


def _build(debug_taps=False):
    key = ("nc", debug_taps)
    if key in _CACHE:
        return _CACHE[key]

    import concourse.bass as bass
    import concourse.tile as tile
    from concourse import bacc, mybir

    f32 = mybir.dt.float32
    bf16 = mybir.dt.bfloat16
    ADD = mybir.AluOpType.add
    MUL = mybir.AluOpType.mult
    EXP = mybir.ActivationFunctionType.Exp
    ts = bass.ts

    nc = bacc.Bacc("TRN2", target_bir_lowering=False, debug=False)

    xt_d = nc.dram_tensor("xt", [D, S], bf16, kind="ExternalInput").ap()
    wqkv_d = nc.dram_tensor("wqkv", [D, FQ + 2 * HD], bf16, kind="ExternalInput").ap()
    wo_d = nc.dram_tensor("wo", [FQ, D], bf16, kind="ExternalInput").ap()
    cos2_d = nc.dram_tensor("cos2", [128, S], f32, kind="ExternalInput").ap()
    sina2_d = nc.dram_tensor("sina2", [128, S], f32, kind="ExternalInput").ap()
    maskt_d = nc.dram_tensor("maskt", [128, 128], f32, kind="ExternalInput").ap()
    idt_d = nc.dram_tensor("idt", [64, 64], bf16, kind="ExternalInput").ap()
    out_d = nc.dram_tensor("out", [S, D], bf16, kind="ExternalOutput").ap()

    if debug_taps:
        dbg_qt = nc.dram_tensor("dbg_qt", [64, NHC, S], bf16, kind="ExternalOutput").ap()
        dbg_kt = nc.dram_tensor("dbg_kt", [64, S], bf16, kind="ExternalOutput").ap()
        dbg_vaug = nc.dram_tensor("dbg_vaug", [128, NCHUNK, 128], bf16, kind="ExternalOutput").ap()
        dbg_at = nc.dram_tensor("dbg_at", [128, 2, S], bf16, kind="ExternalOutput").ap()

    xt_v = xt_d.rearrange("(ko p) s -> p ko s", p=128)
    wqkv_v = wqkv_d.rearrange("(ko p) f -> p ko f", p=128)
    wo_v = wo_d.rearrange("(c p) n -> p c n", p=128)
    out_v = out_d.rearrange("(t p) n -> p t n", p=128)

    with tile.TileContext(nc) as tc:
        with tc.tile_pool(name="consts", bufs=1) as consts, \
             tc.tile_pool(name="persist", bufs=1) as persist:
            wqkv_sb = consts.tile([128, KD, FQ + 2 * HD], bf16, name="wqkv_sb")
            nc.sync.dma_start(wqkv_sb[:], wqkv_v[:])
            wo_sb = consts.tile([128, 2, D], bf16, name="wo_sb")
            nc.sync.dma_start(wo_sb[:], wo_v[:])
            cos2_sb = consts.tile([128, S], f32, name="cos2_sb")
            nc.sync.dma_start(cos2_sb[:], cos2_d[:])
            sina2_sb = consts.tile([128, S], f32, name="sina2_sb")
            nc.sync.dma_start(sina2_sb[:], sina2_d[:])
            maskt_sb = consts.tile([128, 128], f32, name="maskt_sb")
            nc.sync.dma_start(maskt_sb[:], maskt_d[:])
            idt_sb = consts.tile([64, 64], bf16, name="idt_sb")
            nc.sync.dma_start(idt_sb[:], idt_d[:])

            qt_sb = persist.tile([64, NHC, S], bf16, name="qt_sb")
            kt_sb = persist.tile([64, S], bf16, name="kt_sb")
            vaug_sb = persist.tile([128, NCHUNK, 128], bf16, name="vaug_sb")
            at_sb = persist.tile([128, 2, S], bf16, name="at_sb")

            # ones columns first: softmax denominator replicas land on
            # partitions 0:64 of the A^T psum (base-0 for the custom recip op)
            nc.gpsimd.memset(vaug_sb[:, :, 0:64], 1.0)

            # ---------------- Stage A: QKV^T + RoPE + V ----------------
            with tc.tile_pool(name="xtp", bufs=2) as xtp, \
                 tc.tile_pool(name="ropet", bufs=3) as ropet, \
                 tc.tile_pool(name="vtp", bufs=2) as vtp, \
                 tc.tile_pool(name="psA", bufs=3, space="PSUM") as psA, \
                 tc.tile_pool(name="psV", bufs=2, space="PSUM") as psV:
                for sp in range(NSPAN):
                    rng = slice(sp * SPAN, (sp + 1) * SPAN)
                    xts = xtp.tile([128, KD, SPAN], bf16, name="xts")
                    nc.sync.dma_start(xts[:], xt_v[:, :, rng])

                    # q f-tiles (2 heads each) + k tile
                    for ft in range(3):
                        if ft < 2:
                            ps = psA.tile([128, SPAN], f32, name="psqk")
                            pslc = ps[:]
                            fcols = ts(ft, 128)
                            np_lo, np_hi = 128, 64
                        else:
                            ps = psA.tile([128, SPAN], f32, name="psqk")
                            pslc = ps[0:64]
                            fcols = slice(FQ, FQ + HD)
                            np_lo, np_hi = 64, 32
                        for k in range(KD):
                            nc.tensor.matmul(
                                pslc,
                                lhsT=wqkv_sb[:, k, fcols],
                                rhs=xts[:, k, :],
                                start=(k == 0),
                                stop=(k == KD - 1),
                            )
                        t = ropet.tile([128, 2, SPAN], f32, name="ropet")
                        nc.vector.tensor_tensor(
                            t[0:np_lo, 0, :], pslc, cos2_sb[0:np_lo, rng], op=MUL
                        )
                        for o in range(0, np_lo, 64):
                            nc.vector.tensor_tensor(
                                t[o:o + 32, 1, :], ps[o + 32:o + 64],
                                sina2_sb[o:o + 32, rng], op=MUL,
                            )
                            nc.vector.tensor_tensor(
                                t[o + 32:o + 64, 1, :], ps[o:o + 32],
                                sina2_sb[o + 32:o + 64, rng], op=MUL,
                            )
                        if ft < 2:
                            nc.vector.tensor_tensor(
                                qt_sb[:, 2 * ft, rng], t[0:64, 0, :], t[0:64, 1, :], op=ADD
                            )
                            nc.vector.tensor_tensor(
                                qt_sb[:, 2 * ft + 1, rng], t[64:128, 0, :], t[64:128, 1, :], op=ADD
                            )
                        else:
                            nc.vector.tensor_tensor(
                                kt_sb[:, rng], t[0:64, 0, :], t[0:64, 1, :], op=ADD
                            )

                    # v tile: project, then transpose to natural layout
                    psv = psA.tile([128, SPAN], f32, name="psqk")
                    for k in range(KD):
                        nc.tensor.matmul(
                            psv[0:64],
                            lhsT=wqkv_sb[:, k, FQ + HD:FQ + 2 * HD],
                            rhs=xts[:, k, :],
                            start=(k == 0),
                            stop=(k == KD - 1),
                        )
                    vt = vtp.tile([64, SPAN], bf16, name="vt")
                    nc.vector.tensor_copy(vt[:], psv[0:64])
                    for c in range(SPAN // 128):
                        tps = psV.tile([128, 64], bf16, name="tps")
                        nc.tensor.transpose(tps[:], vt[:, ts(c, 128)], idt_sb[:])
                        nc.vector.tensor_copy(
                            vaug_sb[:, sp * 4 + c, 64:128], tps[:]
                        )

            # ------------- Stage B: scores^T, softmax, A^T -------------
            with tc.tile_pool(name="etp", bufs=3) as etp, \
                 tc.tile_pool(name="lp", bufs=3) as lp, \
                 tc.tile_pool(name="psS", bufs=2, space="PSUM") as psS, \
                 tc.tile_pool(name="psAV", bufs=2, space="PSUM") as psAV:
                for sp in range(NSPAN):
                    jmax = 4 * sp + 3
                    for pp in range(2):          # head pairs (0,1) and (2,3)
                        av = psAV.tile([128, 2, SPAN], f32, name="av")
                        for j in range(jmax + 1):
                            lo = max(j * 128 - sp * SPAN, 0)
                            qrng = slice(sp * SPAN + lo, (sp + 1) * SPAN)
                            sc = psS.tile([128, 2, SPAN], f32, name="sc")
                            for hi in range(2):
                                nc.tensor.matmul(
                                    sc[:, hi, lo:SPAN],
                                    lhsT=kt_sb[:, ts(j, 128)],
                                    rhs=qt_sb[:, 2 * pp + hi, qrng],
                                    start=True,
                                    stop=True,
                                )
                            if j >= 4 * sp:      # diagonal chunk -> causal mask
                                for hi in range(2):
                                    nc.vector.tensor_tensor(
                                        sc[:, hi, lo:lo + 128],
                                        sc[:, hi, lo:lo + 128],
                                        maskt_sb[:],
                                        op=ADD,
                                    )
                            et = etp.tile([128, 2, SPAN], bf16, name="et")
                            nc.scalar.activation(et[:, :, lo:SPAN], sc[:, :, lo:SPAN], EXP)
                            for hi in range(2):
                                nc.tensor.matmul(
                                    av[:, hi, lo:SPAN],
                                    lhsT=vaug_sb[:, j, :],
                                    rhs=et[:, hi, lo:SPAN],
                                    start=(j == 0),
                                    stop=(j == jmax),
                                )
                        for hi in range(2):
                            linv = lp.tile([64, SPAN], f32, name="linv")
                            nc.vector.reciprocal_approx_fast(
                                out=linv[:], in_=av[0:64, hi, :]
                            )
                            dst = at_sb[64 * hi:64 * (hi + 1), pp, sp * SPAN:(sp + 1) * SPAN]
                            nc.vector.tensor_tensor(dst, av[64:128, hi, :], linv[:], op=MUL)

            if debug_taps:
                nc.sync.dma_start(dbg_qt[:], qt_sb[:])
                nc.sync.dma_start(dbg_kt[:], kt_sb[:])
                nc.sync.dma_start(dbg_vaug[:], vaug_sb[:])
                nc.sync.dma_start(dbg_at[:], at_sb[:])

            # ---------------- Stage C: output projection ----------------
            with tc.tile_pool(name="psC", bufs=3, space="PSUM") as psC, \
                 tc.tile_pool(name="obp", bufs=3) as obp:
                for st in range(NCHUNK):
                    for no in range(2):
                        pc = psC.tile([128, SPAN], f32, name="pc")
                        for c in range(2):
                            nc.tensor.matmul(
                                pc[:],
                                lhsT=at_sb[:, c, ts(st, 128)],
                                rhs=wo_sb[:, c, ts(no, SPAN)],
                                start=(c == 0),
                                stop=(c == 1),
                            )
                        ob = obp.tile([128, SPAN], bf16, name="ob")
                        nc.any.tensor_copy(ob[:], pc[:])
                        nc.sync.dma_start(out_v[:, st, ts(no, SPAN)], ob[:])

    nc.compile()
    _CACHE[key] = nc
    return nc


def _prep_inputs(x, cos, sin, Wq, Wk, Wv, Wo):
    """Build the 8 per-core input maps (host-side sharding + layout prep)."""
    x = np.asarray(x, np.float32)
    cos = np.asarray(cos, np.float32)
    sin = np.asarray(sin, np.float32)
    Wq = np.asarray(Wq, np.float32) * (1.0 / np.sqrt(HD))  # fold score scale
    Wk = np.asarray(Wk, np.float32)
    Wv = np.asarray(Wv, np.float32)
    Wo = np.asarray(Wo, np.float32)

    cosT = cos.T.copy()                       # [HD, S]
    sinT = sin.T.copy()
    cos2 = np.tile(cosT, (2, 1)).astype(np.float32)          # [128, S]
    sina = np.concatenate([-sinT[0:32], sinT[32:64]], axis=0)
    sina2 = np.tile(sina, (2, 1)).astype(np.float32)         # [128, S]

    p = np.arange(128)[:, None]
    f = np.arange(128)[None, :]
    maskt = np.where(p <= f, 0.0, -1e30).astype(np.float32)  # [sk, sq]
    idt = np.eye(64, dtype=BF16)

    xt = [np.ascontiguousarray(x[b].T).astype(BF16) for b in range(B)]

    in_maps = []
    for c in range(8):
        b, g = divmod(c, 4)
        wqkv = np.concatenate(
            [Wq[:, g * FQ:(g + 1) * FQ],
             Wk[:, g * HD:(g + 1) * HD],
             Wv[:, g * HD:(g + 1) * HD]], axis=1).astype(BF16)
        wo = Wo[g * FQ:(g + 1) * FQ, :].astype(BF16)
        in_maps.append({
            "xt": xt[b],
            "wqkv": wqkv,
            "wo": wo,
            "cos2": cos2,
            "sina2": sina2,
            "maskt": maskt,
            "idt": idt,
        })
    return in_maps


def kernel(x, cos, sin, Wq, Wk, Wv, Wo):
    from concourse.bass_utils import run_bass_kernel_spmd

    nc = _build()
    in_maps = _prep_inputs(x, cos, sin, Wq, Wk, Wv, Wo)
    res = run_bass_kernel_spmd(nc, in_maps, list(range(8)))
    out = np.zeros((B, S, D), np.float32)
    for c in range(8):
        out[c // 4] += res.results[c]["out"].astype(np.float32)
    return out


# revision 7
# speedup vs baseline: 1.1110x; 1.1110x over previous
"""GQA attention (B=2, S=2048, D=1024, H=16, KV=4, HD=64) with RoPE + causal
softmax + output projection, sharded over 8 trn2 NeuronCores.

Sharding: core c -> (b = c // 4, g = c % 4): one batch x one KV group
(4 query heads + 1 kv head) per core.  Wq/Wk/Wv column-sharded, Wo
row-sharded; the all-reduce over Wo row-shards is done on the host
(partial [S, D] outputs summed per batch).

Device kernel (per core, all matmuls bf16 with fp32 PSUM accumulate):
  1. QKV^T = Wqkv^T @ x  via lhsT=Wqkv-chunks, rhs=x^T-chunks (x^T is
     pre-transposed + bf16-cast on host).
  2. RoPE applied in the transposed (head-dim on partitions) layout via
     32-partition-slab multiplies against host-prepared cos/sin tables.
  3. scores^T computed directly (lhsT=k^T chunk, rhs=q^T) so softmax
     needs no transposes: exp on ACT (no max subtraction needed; inputs
     are scaled so |scores| is small), causal handled by trimming +
     a [128,128] additive mask on diagonal blocks.
  4. A^T = V_aug^T-style matmul with lhsT=[V | ones] so the softmax
     denominator accumulates into partitions 64:128 for free.
  5. normalize with reciprocal_approx_fast, then out = A @ Wo chunk,
     DMA'd straight from PSUM to DRAM as the fp32 partial output.
"""

import sys

if "/opt/trn_rl_repo" not in sys.path:
    sys.path.insert(0, "/opt/trn_rl_repo")

import numpy as np
import ml_dtypes

B, S, D = 2, 2048, 1024
H, KV, HD = 16, 4, 64
NHC = H // KV          # query heads per core = 4
FQ = NHC * HD          # 256
SPAN = 512
NSPAN = S // SPAN      # 4
NCHUNK = S // 128      # 16
KD = D // 128          # 8
BF16 = ml_dtypes.bfloat16

_CACHE = {}


def _build(debug_taps=False):
    key = ("nc", debug_taps)
    if key in _CACHE:
        return _CACHE[key]

    import concourse.bass as bass
    import concourse.tile as tile
    from concourse import bacc, mybir

    f32 = mybir.dt.float32
    bf16 = mybir.dt.bfloat16
    ADD = mybir.AluOpType.add
    MUL = mybir.AluOpType.mult
    EXP = mybir.ActivationFunctionType.Exp
    ts = bass.ts

    nc = bacc.Bacc("TRN2", target_bir_lowering=False, debug=False)

    xt_d = nc.dram_tensor("xt", [D, S], bf16, kind="ExternalInput").ap()
    wqkv_d = nc.dram_tensor("wqkv", [D, FQ + 2 * HD], bf16, kind="ExternalInput").ap()
    wo_d = nc.dram_tensor("wo", [FQ, D], bf16, kind="ExternalInput").ap()
    cos2_d = nc.dram_tensor("cos2", [128, S], f32, kind="ExternalInput").ap()
    sina2_d = nc.dram_tensor("sina2", [128, S], f32, kind="ExternalInput").ap()
    maskt_d = nc.dram_tensor("maskt", [128, 128], f32, kind="ExternalInput").ap()
    idt_d = nc.dram_tensor("idt", [64, 64], bf16, kind="ExternalInput").ap()
    out_d = nc.dram_tensor("out", [S, D], bf16, kind="ExternalOutput").ap()

    if debug_taps:
        dbg_qt = nc.dram_tensor("dbg_qt", [64, NHC, S], bf16, kind="ExternalOutput").ap()
        dbg_kt = nc.dram_tensor("dbg_kt", [64, S], bf16, kind="ExternalOutput").ap()
        dbg_vaug = nc.dram_tensor("dbg_vaug", [128, NCHUNK, 128], bf16, kind="ExternalOutput").ap()
        dbg_at = nc.dram_tensor("dbg_at", [128, 2, S], bf16, kind="ExternalOutput").ap()

    xt_v = xt_d.rearrange("(ko p) s -> p ko s", p=128)
    wqkv_v = wqkv_d.rearrange("(ko p) f -> p ko f", p=128)
    wo_v = wo_d.rearrange("(c p) n -> p c n", p=128)
    out_v = out_d.rearrange("(t p) n -> p t n", p=128)

    with tile.TileContext(nc) as tc:
        with tc.tile_pool(name="consts", bufs=1) as consts, \
             tc.tile_pool(name="persist", bufs=1) as persist:
            wqkv_sb = consts.tile([128, KD, FQ + 2 * HD], bf16, name="wqkv_sb")
            nc.sync.dma_start(wqkv_sb[:], wqkv_v[:])
            cos2_sb = consts.tile([128, S], f32, name="cos2_sb")
            nc.sync.dma_start(cos2_sb[:], cos2_d[:])
            sina2_sb = consts.tile([128, S], f32, name="sina2_sb")
            nc.sync.dma_start(sina2_sb[:], sina2_d[:])
            maskt_sb = consts.tile([128, 128], f32, name="maskt_sb")
            nc.sync.dma_start(maskt_sb[:], maskt_d[:])
            idt_sb = consts.tile([64, 64], bf16, name="idt_sb")
            nc.sync.dma_start(idt_sb[:], idt_d[:])

            qt_sb = persist.tile([128, NHC, S], bf16, name="qt_sb")
            kt_sb = persist.tile([128, S], bf16, name="kt_sb")
            vaug_sb = persist.tile([128, NCHUNK, 128], bf16, name="vaug_sb")
            at_sb = persist.tile([128, 2, S], bf16, name="at_sb")

            # ones columns first: softmax denominator replicas land on
            # partitions 0:64 of the A^T psum (base-0 for the custom recip op)
            nc.gpsimd.memset(vaug_sb[:, :, 0:64], 1.0)

            # ---------------- Stage A: QKV^T + RoPE + V ----------------
            with tc.tile_pool(name="xtp", bufs=2) as xtp, \
                 tc.tile_pool(name="ropet", bufs=3) as ropet, \
                 tc.tile_pool(name="vtp", bufs=2) as vtp, \
                 tc.tile_pool(name="psA", bufs=3, space="PSUM") as psA, \
                 tc.tile_pool(name="psV", bufs=2, space="PSUM") as psV:
                for sp in range(NSPAN):
                    rng = slice(sp * SPAN, (sp + 1) * SPAN)
                    xts = xtp.tile([128, KD, SPAN], bf16, name="xts")
                    nc.sync.dma_start(xts[:], xt_v[:, :, rng])

                    # q f-tiles (2 heads each) + k tile
                    for ft in range(3):
                        if ft < 2:
                            ps = psA.tile([128, SPAN], f32, name="psqk")
                            pslc = ps[:]
                            fcols = ts(ft, 128)
                            np_lo, np_hi = 128, 64
                        else:
                            ps = psA.tile([128, SPAN], f32, name="psqk")
                            pslc = ps[0:64]
                            fcols = slice(FQ, FQ + HD)
                            np_lo, np_hi = 64, 32
                        for k in range(KD):
                            nc.tensor.matmul(
                                pslc,
                                lhsT=wqkv_sb[:, k, fcols],
                                rhs=xts[:, k, :],
                                start=(k == 0),
                                stop=(k == KD - 1),
                            )
                        t = ropet.tile([128, 2, SPAN], f32, name="ropet")
                        nc.vector.tensor_tensor(
                            t[0:np_lo, 0, :], pslc, cos2_sb[0:np_lo, rng], op=MUL
                        )
                        for o in range(0, np_lo, 64):
                            nc.vector.tensor_tensor(
                                t[o:o + 32, 1, :], ps[o + 32:o + 64],
                                sina2_sb[o:o + 32, rng], op=MUL,
                            )
                            nc.vector.tensor_tensor(
                                t[o + 32:o + 64, 1, :], ps[o:o + 32],
                                sina2_sb[o + 32:o + 64, rng], op=MUL,
                            )
                        if ft < 2:
                            nc.vector.tensor_tensor(
                                qt_sb[0:64, 2 * ft, rng], t[0:64, 0, :], t[0:64, 1, :], op=ADD
                            )
                            nc.vector.tensor_tensor(
                                qt_sb[0:64, 2 * ft + 1, rng], t[64:128, 0, :], t[64:128, 1, :], op=ADD
                            )
                        else:
                            nc.vector.tensor_tensor(
                                kt_sb[0:64, rng], t[0:64, 0, :], t[0:64, 1, :], op=ADD
                            )

                    # v tile: project, then transpose to natural layout
                    psv = psA.tile([128, SPAN], f32, name="psqk")
                    for k in range(KD):
                        nc.tensor.matmul(
                            psv[0:64],
                            lhsT=wqkv_sb[:, k, FQ + HD:FQ + 2 * HD],
                            rhs=xts[:, k, :],
                            start=(k == 0),
                            stop=(k == KD - 1),
                        )
                    vt = vtp.tile([64, SPAN], bf16, name="vt")
                    nc.vector.tensor_copy(vt[:], psv[0:64])
                    for c in range(SPAN // 128):
                        tps = psV.tile([128, 64], bf16, name="tps")
                        nc.tensor.transpose(tps[:], vt[:, ts(c, 128)], idt_sb[:])
                        nc.vector.tensor_copy(
                            vaug_sb[:, sp * 4 + c, 64:128], tps[:]
                        )

            # duplicate q^T/k^T onto partitions 64:128 so scores matmuls can
            # row-tile two sk-chunks concurrently in disjoint PE row groups
            nc.sync.dma_start(qt_sb[64:128, :, :], qt_sb[0:64, :, :])
            nc.sync.dma_start(kt_sb[64:128, :], kt_sb[0:64, :])

            # ------------- Stage B: scores^T, softmax, A^T -------------
            with tc.tile_pool(name="etp", bufs=3) as etp, \
                 tc.tile_pool(name="lp", bufs=3) as lp, \
                 tc.tile_pool(name="psS", bufs=2, space="PSUM") as psS, \
                 tc.tile_pool(name="psAV", bufs=2, space="PSUM") as psAV:
                for sp in range(NSPAN):
                    jmax = 4 * sp + 3
                    npair = (jmax + 1) // 2
                    for pp in range(2):          # head pairs (0,1) and (2,3)
                        av = psAV.tile([128, 2, SPAN], f32, name="av")
                        for hi in range(2):
                            h = 2 * pp + hi
                            for tpair in range(npair):
                                j0, j1 = 2 * tpair, 2 * tpair + 1
                                lo0 = max(j0 * 128 - sp * SPAN, 0)
                                lo1 = max(j1 * 128 - sp * SPAN, 0)
                                sc = psS.tile([128, 2, SPAN], f32, name="sc")
                                # two sk-chunks concurrently: PE rows 0:63 / 64:127
                                nc.tensor.matmul(
                                    sc[:, 0, lo0:SPAN],
                                    lhsT=kt_sb[0:64, ts(j0, 128)],
                                    rhs=qt_sb[0:64, h, sp * SPAN + lo0:(sp + 1) * SPAN],
                                    start=True,
                                    stop=True,
                                )
                                nc.tensor.matmul(
                                    sc[:, 1, lo1:SPAN],
                                    lhsT=kt_sb[64:128, ts(j1, 128)],
                                    rhs=qt_sb[64:128, h, sp * SPAN + lo1:(sp + 1) * SPAN],
                                    start=True,
                                    stop=True,
                                )
                                for par, j, lo in ((0, j0, lo0), (1, j1, lo1)):
                                    if j >= 4 * sp:   # diagonal chunk -> mask
                                        nc.vector.tensor_tensor(
                                            sc[:, par, lo:lo + 128],
                                            sc[:, par, lo:lo + 128],
                                            maskt_sb[:],
                                            op=ADD,
                                        )
                                et = etp.tile([128, 2, SPAN], bf16, name="et")
                                if lo0 == lo1:
                                    nc.scalar.activation(
                                        et[:, :, lo0:SPAN], sc[:, :, lo0:SPAN], EXP
                                    )
                                else:
                                    nc.scalar.activation(
                                        et[:, 0, lo0:SPAN], sc[:, 0, lo0:SPAN], EXP
                                    )
                                    nc.scalar.activation(
                                        et[:, 1, lo1:SPAN], sc[:, 1, lo1:SPAN], EXP
                                    )
                                for par, j, lo in ((0, j0, lo0), (1, j1, lo1)):
                                    nc.tensor.matmul(
                                        av[:, hi, lo:SPAN],
                                        lhsT=vaug_sb[:, j, :],
                                        rhs=et[:, par, lo:SPAN],
                                        start=(j == 0),
                                        stop=(j == jmax),
                                    )
                        for hi in range(2):
                            linv = lp.tile([64, SPAN], f32, name="linv")
                            nc.vector.reciprocal_approx_fast(
                                out=linv[:], in_=av[0:64, hi, :]
                            )
                            dst = at_sb[64 * hi:64 * (hi + 1), pp, sp * SPAN:(sp + 1) * SPAN]
                            nc.vector.tensor_tensor(dst, av[64:128, hi, :], linv[:], op=MUL)

            if debug_taps:
                nc.sync.dma_start(dbg_qt[:], qt_sb[:])
                nc.sync.dma_start(dbg_kt[:], kt_sb[:])
                nc.sync.dma_start(dbg_vaug[:], vaug_sb[:])
                nc.sync.dma_start(dbg_at[:], at_sb[:])

            # ---------------- Stage C: output projection ----------------
            with tc.tile_pool(name="psC", bufs=3, space="PSUM") as psC, \
                 tc.tile_pool(name="obp", bufs=3) as obp:
                wo_sb = consts.tile([128, 2, D], bf16, name="wo_sb")
                nc.sync.dma_start(wo_sb[:], wo_v[:])
                for st in range(NCHUNK):
                    for no in range(2):
                        pc = psC.tile([128, SPAN], f32, name="pc")
                        for c in range(2):
                            nc.tensor.matmul(
                                pc[:],
                                lhsT=at_sb[:, c, ts(st, 128)],
                                rhs=wo_sb[:, c, ts(no, SPAN)],
                                start=(c == 0),
                                stop=(c == 1),
                            )
                        ob = obp.tile([128, SPAN], bf16, name="ob")
                        nc.vector.tensor_copy(ob[:], pc[:])
                        nc.sync.dma_start(out_v[:, st, ts(no, SPAN)], ob[:])

    nc.compile()
    _CACHE[key] = nc
    return nc


def _prep_inputs(x, cos, sin, Wq, Wk, Wv, Wo):
    """Build the 8 per-core input maps (host-side sharding + layout prep)."""
    x = np.asarray(x, np.float32)
    cos = np.asarray(cos, np.float32)
    sin = np.asarray(sin, np.float32)
    Wq = np.asarray(Wq, np.float32) * (1.0 / np.sqrt(HD))  # fold score scale
    Wk = np.asarray(Wk, np.float32)
    Wv = np.asarray(Wv, np.float32)
    Wo = np.asarray(Wo, np.float32)

    cosT = cos.T.copy()                       # [HD, S]
    sinT = sin.T.copy()
    cos2 = np.tile(cosT, (2, 1)).astype(np.float32)          # [128, S]
    sina = np.concatenate([-sinT[0:32], sinT[32:64]], axis=0)
    sina2 = np.tile(sina, (2, 1)).astype(np.float32)         # [128, S]

    p = np.arange(128)[:, None]
    f = np.arange(128)[None, :]
    maskt = np.where(p <= f, 0.0, -1e30).astype(np.float32)  # [sk, sq]
    idt = np.eye(64, dtype=BF16)

    xt = [np.ascontiguousarray(x[b].T).astype(BF16) for b in range(B)]

    in_maps = []
    for c in range(8):
        b, g = divmod(c, 4)
        wqkv = np.concatenate(
            [Wq[:, g * FQ:(g + 1) * FQ],
             Wk[:, g * HD:(g + 1) * HD],
             Wv[:, g * HD:(g + 1) * HD]], axis=1).astype(BF16)
        wo = Wo[g * FQ:(g + 1) * FQ, :].astype(BF16)
        in_maps.append({
            "xt": xt[b],
            "wqkv": wqkv,
            "wo": wo,
            "cos2": cos2,
            "sina2": sina2,
            "maskt": maskt,
            "idt": idt,
        })
    return in_maps


def kernel(x, cos, sin, Wq, Wk, Wv, Wo):
    from concourse.bass_utils import run_bass_kernel_spmd

    nc = _build()
    in_maps = _prep_inputs(x, cos, sin, Wq, Wk, Wv, Wo)
    res = run_bass_kernel_spmd(nc, in_maps, list(range(8)))
    out = np.zeros((B, S, D), np.float32)
    for c in range(8):
        out[c // 4] += res.results[c]["out"].astype(np.float32)
    return out


# revision 8
# speedup vs baseline: 1.3022x; 1.1721x over previous
"""GQA attention (B=2, S=2048, D=1024, H=16, KV=4, HD=64) with RoPE + causal
softmax + output projection, sharded over 8 trn2 NeuronCores.

Sharding: core c -> (b = c // 4, g = c % 4): one batch x one KV group
(4 query heads + 1 kv head) per core.  Wq/Wk/Wv column-sharded, Wo
row-sharded; the all-reduce over Wo row-shards is done on the host
(partial [S, D] outputs summed per batch).

Device kernel (per core, all matmuls bf16 with fp32 PSUM accumulate):
  1. QKV^T = Wqkv^T @ x  via lhsT=Wqkv-chunks, rhs=x^T-chunks (x^T is
     pre-transposed + bf16-cast on host).
  2. RoPE applied in the transposed (head-dim on partitions) layout via
     32-partition-slab multiplies against host-prepared cos/sin tables.
  3. scores^T computed directly (lhsT=k^T chunk, rhs=q^T) so softmax
     needs no transposes: exp on ACT (no max subtraction needed; inputs
     are scaled so |scores| is small), causal handled by trimming +
     a [128,128] additive mask on diagonal blocks.
  4. A^T = V_aug^T-style matmul with lhsT=[V | ones] so the softmax
     denominator accumulates into partitions 64:128 for free.
  5. normalize with reciprocal_approx_fast, then out = A @ Wo chunk,
     DMA'd straight from PSUM to DRAM as the fp32 partial output.
"""

import sys

if "/opt/trn_rl_repo" not in sys.path:
    sys.path.insert(0, "/opt/trn_rl_repo")

import numpy as np
import ml_dtypes

B, S, D = 2, 2048, 1024
H, KV, HD = 16, 4, 64
NHC = H // KV          # query heads per core = 4
FQ = NHC * HD          # 256
SPAN = 512
NSPAN = S // SPAN      # 4
NCHUNK = S // 128      # 16
KD = D // 128          # 8
BF16 = ml_dtypes.bfloat16

_CACHE = {}


def _build(debug_taps=False):
    key = ("nc", debug_taps)
    if key in _CACHE:
        return _CACHE[key]

    import concourse.bass as bass
    import concourse.tile as tile
    from concourse import bacc, mybir

    f32 = mybir.dt.float32
    bf16 = mybir.dt.bfloat16
    ADD = mybir.AluOpType.add
    MUL = mybir.AluOpType.mult
    EXP = mybir.ActivationFunctionType.Exp
    ts = bass.ts

    nc = bacc.Bacc("TRN2", target_bir_lowering=False, debug=False)

    xt_d = nc.dram_tensor("xt", [D, S], bf16, kind="ExternalInput").ap()
    wqkv_d = nc.dram_tensor("wqkv", [D, FQ + 2 * HD], bf16, kind="ExternalInput").ap()
    wo_d = nc.dram_tensor("wo", [FQ, D], bf16, kind="ExternalInput").ap()
    cos2_d = nc.dram_tensor("cos2", [128, S], f32, kind="ExternalInput").ap()
    sina2_d = nc.dram_tensor("sina2", [128, S], f32, kind="ExternalInput").ap()
    maskt_d = nc.dram_tensor("maskt", [128, 128], bf16, kind="ExternalInput").ap()
    idt_d = nc.dram_tensor("idt", [64, 64], bf16, kind="ExternalInput").ap()
    out_d = nc.dram_tensor("out", [S, D], bf16, kind="ExternalOutput").ap()

    if debug_taps:
        dbg_qt = nc.dram_tensor("dbg_qt", [64, NHC, S], bf16, kind="ExternalOutput").ap()
        dbg_kt = nc.dram_tensor("dbg_kt", [64, S], bf16, kind="ExternalOutput").ap()
        dbg_vaug = nc.dram_tensor("dbg_vaug", [128, NCHUNK, 128], bf16, kind="ExternalOutput").ap()
        dbg_at = nc.dram_tensor("dbg_at", [128, 2, S], bf16, kind="ExternalOutput").ap()

    xt_v = xt_d.rearrange("(ko p) s -> p ko s", p=128)
    wqkv_v = wqkv_d.rearrange("(ko p) f -> p ko f", p=128)
    wo_v = wo_d.rearrange("(c p) n -> p c n", p=128)
    out_v = out_d.rearrange("(t p) n -> p t n", p=128)

    with tile.TileContext(nc) as tc:
        with tc.tile_pool(name="consts", bufs=1) as consts, \
             tc.tile_pool(name="persist", bufs=1) as persist:
            wqkv_sb = consts.tile([128, KD, FQ + 2 * HD], bf16, name="wqkv_sb")
            nc.sync.dma_start(wqkv_sb[:], wqkv_v[:])
            cos2_sb = consts.tile([128, S], f32, name="cos2_sb")
            nc.sync.dma_start(cos2_sb[:], cos2_d[:])
            sina2_sb = consts.tile([128, S], f32, name="sina2_sb")
            nc.sync.dma_start(sina2_sb[:], sina2_d[:])
            maskt_sb = consts.tile([128, 128], bf16, name="maskt_sb")
            nc.sync.dma_start(maskt_sb[:], maskt_d[:])
            idt_sb = consts.tile([64, 64], bf16, name="idt_sb")
            nc.sync.dma_start(idt_sb[:], idt_d[:])

            qt_sb = persist.tile([128, NHC, S], bf16, name="qt_sb")
            kt_sb = persist.tile([128, S], bf16, name="kt_sb")
            vaug_sb = persist.tile([128, NCHUNK, 128], bf16, name="vaug_sb")
            at_sb = persist.tile([128, 2, S], bf16, name="at_sb")

            # ones columns first: softmax denominator replicas land on
            # partitions 0:64 of the A^T psum (base-0 for the custom recip op)
            nc.gpsimd.memset(vaug_sb[:, :, 0:64], 1.0)

            # ---------------- Stage A: QKV^T + RoPE + V ----------------
            with tc.tile_pool(name="xtp", bufs=2) as xtp, \
                 tc.tile_pool(name="ropet", bufs=3) as ropet, \
                 tc.tile_pool(name="vtp", bufs=4) as vtp, \
                 tc.tile_pool(name="psA", bufs=3, space="PSUM") as psA, \
                 tc.tile_pool(name="psV", bufs=2, space="PSUM") as psV:
                vts = []
                for sp in range(NSPAN):
                    rng = slice(sp * SPAN, (sp + 1) * SPAN)
                    xts = xtp.tile([128, KD, SPAN], bf16, name="xts")
                    nc.sync.dma_start(xts[:], xt_v[:, :, rng])

                    # q f-tiles (2 heads each) + k tile
                    for ft in range(3):
                        if ft < 2:
                            ps = psA.tile([128, SPAN], f32, name="psqk")
                            pslc = ps[:]
                            fcols = ts(ft, 128)
                            np_lo, np_hi = 128, 64
                        else:
                            ps = psA.tile([128, SPAN], f32, name="psqk")
                            pslc = ps[0:64]
                            fcols = slice(FQ, FQ + HD)
                            np_lo, np_hi = 64, 32
                        for k in range(KD):
                            nc.tensor.matmul(
                                pslc,
                                lhsT=wqkv_sb[:, k, fcols],
                                rhs=xts[:, k, :],
                                start=(k == 0),
                                stop=(k == KD - 1),
                            )
                        t = ropet.tile([128, 2, SPAN], f32, name="ropet")
                        nc.vector.tensor_tensor(
                            t[0:np_lo, 0, :], pslc, cos2_sb[0:np_lo, rng], op=MUL
                        )
                        for o in range(0, np_lo, 64):
                            nc.vector.tensor_tensor(
                                t[o:o + 32, 1, :], ps[o + 32:o + 64],
                                sina2_sb[o:o + 32, rng], op=MUL,
                            )
                            nc.vector.tensor_tensor(
                                t[o + 32:o + 64, 1, :], ps[o:o + 32],
                                sina2_sb[o + 32:o + 64, rng], op=MUL,
                            )
                        if ft < 2:
                            nc.gpsimd.tensor_tensor(
                                qt_sb[0:64, 2 * ft, rng], t[0:64, 0, :], t[0:64, 1, :], op=ADD
                            )
                            nc.gpsimd.tensor_tensor(
                                qt_sb[0:64, 2 * ft + 1, rng], t[64:128, 0, :], t[64:128, 1, :], op=ADD
                            )
                        else:
                            nc.gpsimd.tensor_tensor(
                                kt_sb[0:64, rng], t[0:64, 0, :], t[0:64, 1, :], op=ADD
                            )

                    # v tile: project, then transpose to natural layout
                    psv = psA.tile([128, SPAN], f32, name="psqk")
                    for k in range(KD):
                        nc.tensor.matmul(
                            psv[0:64],
                            lhsT=wqkv_sb[:, k, FQ + HD:FQ + 2 * HD],
                            rhs=xts[:, k, :],
                            start=(k == 0),
                            stop=(k == KD - 1),
                        )
                    vt = vtp.tile([64, SPAN], bf16, name=f"vt{sp}", tag=f"vt{sp}")
                    nc.vector.tensor_copy(vt[:], psv[0:64])
                    vts.append(vt)

                    # duplicate this span of q^T/k^T onto partitions 64:128 so
                    # scores matmuls can row-tile two sk-chunks concurrently
                    nc.sync.dma_start(qt_sb[64:128, :, rng], qt_sb[0:64, :, rng])
                    nc.sync.dma_start(kt_sb[64:128, rng], kt_sb[0:64, rng])

                # batched V transposes (keeps PE stream dense inside the span loop)
                for sp in range(NSPAN):
                    for c in range(SPAN // 128):
                        tps = psV.tile([128, 64], bf16, name="tps")
                        nc.tensor.transpose(tps[:], vts[sp][:, ts(c, 128)], idt_sb[:])
                        nc.vector.tensor_copy(
                            vaug_sb[:, sp * 4 + c, 64:128], tps[:]
                        )

            # ------------- Stage B: scores^T, softmax, A^T -------------
            with tc.tile_pool(name="etp", bufs=3) as etp, \
                 tc.tile_pool(name="lp", bufs=3) as lp, \
                 tc.tile_pool(name="psS", bufs=2, space="PSUM") as psS, \
                 tc.tile_pool(name="psAV", bufs=1, space="PSUM") as psAV, \
                 tc.tile_pool(name="psC", bufs=2, space="PSUM") as psC, \
                 tc.tile_pool(name="obp", bufs=3) as obp:
                wo_sb = consts.tile([128, 2, D], bf16, name="wo_sb")
                nc.sync.dma_start(wo_sb[:], wo_v[:])
                for sp in range(NSPAN):
                    jmax = 4 * sp + 3
                    npair = (jmax + 1) // 2
                    for pp in range(2):          # head pairs (0,1) and (2,3)
                        av = psAV.tile([128, 2, SPAN], f32, name="av")
                        for hi in range(2):
                            h = 2 * pp + hi
                            for tpair in range(npair):
                                j0, j1 = 2 * tpair, 2 * tpair + 1
                                lo0 = max(j0 * 128 - sp * SPAN, 0)
                                lo1 = max(j1 * 128 - sp * SPAN, 0)
                                sc = psS.tile([128, 2, SPAN], f32, name="sc")
                                # two sk-chunks concurrently: PE rows 0:63 / 64:127
                                nc.tensor.matmul(
                                    sc[:, 0, lo0:SPAN],
                                    lhsT=kt_sb[0:64, ts(j0, 128)],
                                    rhs=qt_sb[0:64, h, sp * SPAN + lo0:(sp + 1) * SPAN],
                                    start=True,
                                    stop=True,
                                )
                                nc.tensor.matmul(
                                    sc[:, 1, lo1:SPAN],
                                    lhsT=kt_sb[64:128, ts(j1, 128)],
                                    rhs=qt_sb[64:128, h, sp * SPAN + lo1:(sp + 1) * SPAN],
                                    start=True,
                                    stop=True,
                                )
                                et = etp.tile([128, 2, SPAN], bf16, name="et")
                                if lo0 == lo1:
                                    nc.scalar.activation(
                                        et[:, :, lo0:SPAN], sc[:, :, lo0:SPAN], EXP
                                    )
                                else:
                                    nc.scalar.activation(
                                        et[:, 0, lo0:SPAN], sc[:, 0, lo0:SPAN], EXP
                                    )
                                    nc.scalar.activation(
                                        et[:, 1, lo1:SPAN], sc[:, 1, lo1:SPAN], EXP
                                    )
                                for par, j, lo in ((0, j0, lo0), (1, j1, lo1)):
                                    if j >= 4 * sp:   # diagonal chunk -> 0/1 mask
                                        nc.gpsimd.tensor_tensor(
                                            et[:, par, lo:lo + 128],
                                            et[:, par, lo:lo + 128],
                                            maskt_sb[:],
                                            op=MUL,
                                        )
                                for par, j, lo in ((0, j0, lo0), (1, j1, lo1)):
                                    nc.tensor.matmul(
                                        av[:, hi, lo:SPAN],
                                        lhsT=vaug_sb[:, j, :],
                                        rhs=et[:, par, lo:SPAN],
                                        start=(j == 0),
                                        stop=(j == jmax),
                                    )
                        for hi in range(2):
                            linv = lp.tile([64, SPAN], f32, name="linv")
                            nc.vector.reciprocal_approx_fast(
                                out=linv[:], in_=av[0:64, hi, :]
                            )
                            dst = at_sb[64 * hi:64 * (hi + 1), pp, sp * SPAN:(sp + 1) * SPAN]
                            nc.vector.tensor_tensor(dst, av[64:128, hi, :], linv[:], op=MUL)

                    # output projection for this span (overlaps next span's B work)
                    for st in range(4 * sp, 4 * sp + 4):
                        for no in range(2):
                            pc = psC.tile([128, SPAN], f32, name="pc")
                            for c in range(2):
                                nc.tensor.matmul(
                                    pc[:],
                                    lhsT=at_sb[:, c, ts(st, 128)],
                                    rhs=wo_sb[:, c, ts(no, SPAN)],
                                    start=(c == 0),
                                    stop=(c == 1),
                                )
                            ob = obp.tile([128, SPAN], bf16, name="ob")
                            nc.vector.tensor_copy(ob[:], pc[:])
                            nc.sync.dma_start(out_v[:, st, ts(no, SPAN)], ob[:])

            if debug_taps:
                nc.sync.dma_start(dbg_qt[:], qt_sb[:])
                nc.sync.dma_start(dbg_kt[:], kt_sb[:])
                nc.sync.dma_start(dbg_vaug[:], vaug_sb[:])
                nc.sync.dma_start(dbg_at[:], at_sb[:])


    nc.compile()
    _CACHE[key] = nc
    return nc


def _prep_inputs(x, cos, sin, Wq, Wk, Wv, Wo):
    """Build the 8 per-core input maps (host-side sharding + layout prep)."""
    x = np.asarray(x, np.float32)
    cos = np.asarray(cos, np.float32)
    sin = np.asarray(sin, np.float32)
    Wq = np.asarray(Wq, np.float32) * (1.0 / np.sqrt(HD))  # fold score scale
    Wk = np.asarray(Wk, np.float32)
    Wv = np.asarray(Wv, np.float32)
    Wo = np.asarray(Wo, np.float32)

    cosT = cos.T.copy()                       # [HD, S]
    sinT = sin.T.copy()
    cos2 = np.tile(cosT, (2, 1)).astype(np.float32)          # [128, S]
    sina = np.concatenate([-sinT[0:32], sinT[32:64]], axis=0)
    sina2 = np.tile(sina, (2, 1)).astype(np.float32)         # [128, S]

    p = np.arange(128)[:, None]
    f = np.arange(128)[None, :]
    maskt = np.where(p <= f, 1.0, 0.0).astype(BF16)          # [sk, sq]
    idt = np.eye(64, dtype=BF16)

    xt = [np.ascontiguousarray(x[b].T).astype(BF16) for b in range(B)]

    in_maps = []
    for c in range(8):
        b, g = divmod(c, 4)
        wqkv = np.concatenate(
            [Wq[:, g * FQ:(g + 1) * FQ],
             Wk[:, g * HD:(g + 1) * HD],
             Wv[:, g * HD:(g + 1) * HD]], axis=1).astype(BF16)
        wo = Wo[g * FQ:(g + 1) * FQ, :].astype(BF16)
        in_maps.append({
            "xt": xt[b],
            "wqkv": wqkv,
            "wo": wo,
            "cos2": cos2,
            "sina2": sina2,
            "maskt": maskt,
            "idt": idt,
        })
    return in_maps


def kernel(x, cos, sin, Wq, Wk, Wv, Wo):
    from concourse.bass_utils import run_bass_kernel_spmd

    nc = _build()
    in_maps = _prep_inputs(x, cos, sin, Wq, Wk, Wv, Wo)
    res = run_bass_kernel_spmd(nc, in_maps, list(range(8)))
    out = np.zeros((B, S, D), np.float32)
    for c in range(8):
        out[c // 4] += res.results[c]["out"].astype(np.float32)
    return out
